# revision 10
# baseline (speedup 1.0000x reference)
"""Trainium2 Bass kernel v9 for nn_AttentionLayer.

Math (per core, vocab-sharded): out[b, v'] = occ[b, v'] * leaky_relu(t[v'] + s[b])
with t = table_shard^T a_w (PE, bf16), s = attr_emb @ a_a (DVE, f32).

v8 vs v7 (40.9us). v7 post-mortem: the scalar engine's 5th DMA dispatch
waited ~7us for a HWDGE ring slot and stalled the whole ACT chain queued
behind it; cold-PE matmuls (630ns vs 375ns warm) paced the strips; and
out-dispatches sat between ACTIVATEs. Now:
  - scalar ring carries ONLY aa/attr/awb (3 tiny loads, under the ring
    capacity) then the pure ACT chain; every other DMA lives on sync.
  - sync ring order: per-strip table pairs interleaved with occ chunks,
    then per-(strip,h) stores.
  - PE warm-up: 11 dummy matmuls (no input deps) so HAM is at full clock
    when the real accumulations start; warm-up PSUM shares the pt pool.
  - 4 strips of 1600 instead of 5x1280: fewer ACT/TT ops (less fixed
    overhead), pt = 4 PSUM banks x 2 bufs = all 8 banks.
HBM/core: tbl 3.28 + occ 1.64 (i8; DMA is the saturated critical path,
so fewer bytes beat DVE 2x mode) + out 3.28 = 8.2 MB.
"""

import numpy as np
import ml_dtypes

import concourse.bass as bass
import concourse.tile as tile
from concourse import bacc, mybir
from concourse.bass_utils import run_bass_kernel_spmd

B = 256
L = 512
V = 50257
DW = 256
DA = 256
ALPHA = 0.2

NCORES = 8
VS = 6400          # vocab span per core
SW = 1600          # strip width
NS = VS // SW      # 4 strips
OG = 2 * SW        # occ chunk width (2 strips)

BF16 = ml_dtypes.bfloat16

_CACHE = {}


def _build():
    if "nc" in _CACHE:
        return _CACHE["nc"]
    f32 = mybir.dt.float32
    bf16 = mybir.dt.bfloat16

    nc = bacc.Bacc("TRN2", target_bir_lowering=False, debug=False)
    tbl = nc.declare_dram_parameter("tbl", [128, 2 * VS], bf16, isOutput=False)
    occ = nc.declare_dram_parameter("occ", [128, 2 * VS], mybir.dt.int8, isOutput=False)
    awb = nc.declare_dram_parameter("awb", [128, 2 * 128], bf16, isOutput=False)
    aa = nc.declare_dram_parameter("aa", [128, DA], f32, isOutput=False)
    attr = nc.declare_dram_parameter("attr", [128, 2 * DA], f32, isOutput=False)
    out = nc.declare_dram_parameter("out", [128, 2 * VS], bf16, isOutput=True)

    NCH = ((0, 512), (512, 1024), (1024, 1536), (1536, SW))

    with tile.TileContext(nc) as tc:
        with (
            tc.tile_pool(name="sb", bufs=1) as sb,
            tc.tile_pool(name="pst", bufs=2, space="PSUM") as pst,
        ):
            # ---- tiny loads on the scalar ring (stays under ring capacity,
            # so the ACT chain behind them never stalls on a ring slot) ----
            aa_t = sb.tile([128, DA], f32, tag="aa")
            nc.scalar.dma_start(aa_t[:], aa.ap())
            at = sb.tile([128, 2 * DA], f32, tag="attr")
            nc.scalar.dma_start(at[:], attr.ap())
            awb_t = sb.tile([128, 2 * 128], bf16, tag="awb")
            nc.scalar.dma_start(awb_t[:], awb.ap())

            # ---- PE warm-up: dummy matmuls with no input deps; shares the
            # pt pool so PSUM stays within 8 banks ----
            wres = sb.tile([128, 512], bf16, tag="wres")
            nc.vector.memset(wres[:], 0.0)
            wpt = pst.tile([128, SW], f32, tag="pt", name="wpt")
            for wi in range(11):
                nc.tensor.matmul(
                    wpt[:, 0:512],
                    lhsT=wres[:, 0:128],
                    rhs=wres[:],
                    start=True,
                    stop=True,
                )

            # ---- sync ring: strip tables interleaved with occ chunks ----
            ts = {}
            ocg = [[None, None], [None, None]]  # [h][grp]
            for si in range(NS):
                for dh in range(2):
                    t_ = sb.tile([128, SW], bf16, tag=f"t{si}{dh}", name=f"t{si}{dh}")
                    nc.sync.dma_start(
                        t_[:],
                        tbl.ap()[:, dh * VS + si * SW : dh * VS + (si + 1) * SW],
                    )
                    ts[(si, dh)] = t_
                if si % 2 == 1:
                    grp = si // 2
                    c0 = grp * OG
                    for h in range(2):
                        o_ = sb.tile([128, OG], mybir.dt.int8, tag=f"oc{h}{grp}", name=f"oc{h}{grp}")
                        nc.sync.dma_start(
                            o_[:], occ.ap()[:, h * VS + c0 : h * VS + c0 + OG]
                        )
                        ocg[h][grp] = o_

            # ---- s = attr_emb @ a_a  (s_sb[:, h] holds b = h*128 + p) ----
            s_sb = sb.tile([128, 2], f32, tag="s")
            for h in range(2):
                pa = sb.tile([128, DA], f32, tag=f"pa{h}")
                nc.vector.tensor_tensor(
                    out=pa[:],
                    in0=at[:, h * DA : (h + 1) * DA],
                    in1=aa_t[:],
                    op=mybir.AluOpType.mult,
                )
                nc.vector.tensor_reduce(
                    out=s_sb[:, h : h + 1],
                    in_=pa[:],
                    axis=mybir.AxisListType.X,
                    op=mybir.AluOpType.add,
                )

            # ---- per strip: matmul + both ACT passes, then mask + store ----
            for si in range(NS):
                grp = si // 2
                off = (si % 2) * SW
                pt = pst.tile([128, SW], f32, tag="pt")
                for dh in range(2):
                    for n0, n1 in NCH:
                        nc.tensor.matmul(
                            pt[:, n0:n1],
                            lhsT=awb_t[:, dh * 128 : (dh + 1) * 128],
                            rhs=ts[(si, dh)][:, n0:n1],
                            start=(dh == 0),
                            stop=(dh == 1),
                        )
                for h in range(2):
                    o1 = sb.tile([128, SW], bf16, tag=f"o1_{si}_{h}", name=f"o1_{si}_{h}")
                    nc.scalar.activation(
                        o1[:],
                        pt[:],
                        mybir.ActivationFunctionType.Prelu,
                        bias=s_sb[:, h : h + 1],
                        scale=1.0,
                        alpha=ALPHA,
                    )
                    o = sb.tile([128, SW], bf16, tag=f"o_{si}_{h}", name=f"o_{si}_{h}")
                    nc.vector.tensor_tensor(
                        out=o[:],
                        in0=o1[:],
                        in1=ocg[h][grp][:, off : off + SW],
                        op=mybir.AluOpType.mult,
                    )
                    nc.sync.dma_start(
                        out.ap()[:, h * VS + si * SW : h * VS + (si + 1) * SW],
                        o[:],
                    )

    nc.compile()
    _CACHE["nc"] = nc
    return nc


def _pmaj(x):
    """[256, N] -> partition-major [128, 2*N] (halves along columns)."""
    n = x.shape[1]
    return np.ascontiguousarray(
        x.reshape(2, 128, n).transpose(1, 0, 2).reshape(128, 2 * n)
    )


def _prep_inputs(words, word_emb_table, attr_emb, a):
    words = np.ascontiguousarray(words).astype(np.int64)
    wet = np.ascontiguousarray(word_emb_table, dtype=np.float32)
    attr = np.ascontiguousarray(attr_emb, dtype=np.float32)
    a = np.ascontiguousarray(a, dtype=np.float32).reshape(-1)

    # awb_dev[p, dh*128 + m] = a[dh*128 + p]
    A = a[:DW].astype(BF16).reshape(2, 128)
    awb_dev = np.ascontiguousarray(
        np.repeat(A.T[:, :, None], 128, axis=2).reshape(128, 2 * 128)
    )
    aa_rep = np.ascontiguousarray(np.broadcast_to(a[DW:][None, :], (128, DA)))
    attr_dev = _pmaj(attr)

    tblpad = np.zeros((NCORES * VS, DW), dtype=np.float32)
    tblpad[:V] = wet
    tbl_bf = tblpad.astype(BF16)

    occ_full = np.zeros((B, NCORES * VS), dtype=np.int8)
    rows = np.repeat(np.arange(B), L)
    occ_full[rows, words.reshape(-1)] = 1

    in_maps = []
    for i in range(NCORES):
        blk = tbl_bf[i * VS : (i + 1) * VS, :]          # [VS, 256]
        tbl_dev = _pmaj(np.ascontiguousarray(blk.T))    # [128, 2*VS]
        occ_dev = _pmaj(occ_full[:, i * VS : (i + 1) * VS])
        in_maps.append(
            {
                "tbl": tbl_dev,
                "occ": occ_dev,
                "awb": awb_dev,
                "aa": aa_rep,
                "attr": attr_dev,
            }
        )
    return in_maps


def kernel(words, word_emb_table, attr_emb, a, _trace=False, **_kw):
    nc = _build()
    in_maps = _prep_inputs(words, word_emb_table, attr_emb, a)
    res = run_bass_kernel_spmd(nc, in_maps, list(range(NCORES)), trace=_trace)
    parts = []
    for i in range(NCORES):
        o = res.results[i]["out"]                       # [128, 2*VS] bf16
        parts.append(o.reshape(128, 2, VS).transpose(1, 0, 2).reshape(B, VS))
    out = np.ascontiguousarray(
        np.concatenate(parts, axis=1)[:, :V].astype(np.float32)
    )
    if _trace:
        return out, res
    return out
